# revision 1
# baseline (speedup 1.0000x reference)
"""Trainium2 Bass kernel for nn_BinConv2d (XNOR-style binary conv block).

Reference computation (per the oracle):
  h     = batchnorm(x; batch stats over (N,H,W), eps=1e-4, gamma, beta)
  x_bin = sign(h)
  c     = clip(w - mean_c(w), -1, 1); w_bin = sign(c); m_o = mean|c|
  y     = relu((conv2d(x_bin, w_bin, pad=1) + bias) * m_o)

Strategy: data-parallel over batch (4 images per core, 8 cores).
BN statistics are computed on-device per core and combined with a tiny
AllReduce. Both conv operands are exactly +-1 (or 0), so the conv is
computed exactly in fp8e4 with fp32 PSUM accumulation, using DoubleRow
perf mode to contract K=256 per matmul. The 3x3 conv is expressed as 9
shifted accumulating matmuls over a zero-padded SBUF image layout.
"""

import os
import sys

import numpy as np

_TRN_REPO = "/opt/trn_rl_repo"
if _TRN_REPO not in sys.path:
    sys.path.insert(0, _TRN_REPO)

import concourse.bass as bass
import concourse.mybir as mybir
import concourse.tile as tile
from concourse.masks import make_identity

P = 128
C = 256
O = 256
H = W = 56
HW = H * W            # 3136
KH = KW = 3
NKK = KH * KW         # 9
CK = C * NKK          # 2304
RG = 58               # padded row width (1 + 56 + 1)
RPI = 58              # padded rows per image
N_TOTAL = 32
N_CORES = 8
CNT = N_TOTAL * HW    # BN reduction count per channel
BN_EPS = 1e-4
TROWS = 8             # padded output rows per psum tile
NT = H // TROWS       # 7 tiles per image
FREE = TROWS * RG     # 464 (contiguous padded-flat columns, incl pad cols)
OFREE = TROWS * W     # 448 (valid output columns per psum tile)

F32 = mybir.dt.float32
ALU = mybir.AluOpType
AF = mybir.ActivationFunctionType
AX = mybir.AxisListType


def _legalize_sync_waits(nc, max_waits: int = 1):
    """Work around the ISA's tiny per-instruction sync-wait budgets.

    Tile emits as many semaphore waits per instruction as the dependency
    graph needs, but most walrus instruction formats encode only one sync
    wait ("Too many sync wait commands" codegen failure otherwise).

    Two transformations, both semantics-preserving:
    1. Drop same-engine self-waits that are trivially satisfied: engines
       retire instructions in order, so a wait on the instruction's own
       engine semaphore for a value already reached by preceding
       same-engine updates is a no-op.
    2. For instructions still exceeding `max_waits`, insert a same-engine
       Drain immediately before the offender carrying the excess waits —
       an identical blocking point on the same engine queue (the
       end-of-kernel drain routinely carries 13+ waits, so the Drain
       format is known to have capacity).
    """
    import re

    import bass_rust

    eng_builder = {
        mybir.EngineType.PE: nc.tensor,
        mybir.EngineType.DVE: nc.vector,
        mybir.EngineType.Activation: nc.scalar,
        mybir.EngineType.Pool: nc.gpsimd,
        mybir.EngineType.SP: nc.sync,
    }
    # Same-engine self-wait elision: engines issue in order, but elements
    # pipeline, so a RAW against the *immediately* preceding instruction
    # still needs its wait. A self-wait whose producer retired many
    # instructions ago is dead weight, and these are what blow the 1-slot
    # budget in the hot loop (each displaced wait otherwise becomes a
    # Drain, ~0.4-1.6us of engine stall). Elide only when the producer is
    # at least `margin` same-engine updates in the past.
    margin = 8
    self_pat = {
        mybir.EngineType.PE: re.compile(r"^PE_\d+$"),
        mybir.EngineType.DVE: re.compile(r"^DVE_\d+$"),
        mybir.EngineType.Activation: re.compile(r"^Activation_\d+$"),
    }

    def make_drain(engine):
        counts = {id(b): len(b.instructions) for b in nc.main_func.blocks}
        eng_builder[engine].drain()
        for b in nc.main_func.blocks:
            if len(b.instructions) != counts[id(b)]:
                return b.instructions.pop()
        raise RuntimeError("drain emission not found")

    upd: dict = {}
    n_elided = n_moved = 0
    for bb in nc.main_func.blocks:
        out = []
        for ins in bb.instructions:
            si = ins.sync_info
            if si is not None and si.on_wait:
                pat = self_pat.get(ins.engine)
                keep = []
                for w in si.on_wait:
                    if (
                        pat is not None
                        and w.sync_type == "semaphore"
                        and w.wait_mode == "sem-ge-imm"
                        and pat.match(w.ant_name)
                        and upd.get(w.ant_name, 0) >= (w.wait_value or 0) + margin
                    ):
                        n_elided += 1
                        continue
                    keep.append(w)
                while len(keep) > max_waits:
                    dr = make_drain(ins.engine)
                    dr.sync_info = bass_rust.SyncInfo(
                        on_wait=[keep.pop(0)], on_update=[]
                    )
                    out.append(dr)
                    n_moved += 1
                if len(keep) != len(si.on_wait):
                    ins.sync_info = bass_rust.SyncInfo(
                        on_wait=keep, on_update=list(si.on_update)
                    )
            si2 = ins.sync_info
            if si2 is not None:
                for u in si2.on_update:
                    if u.update_mode == "sem-inc":
                        upd[u.ant_name] = upd.get(u.ant_name, 0) + (
                            u.update_value or 1
                        )
            out.append(ins)
        bb.instructions[:] = out
    return n_elided, n_moved


def build_program(nl: int, n_cores: int, use_fp8: bool,
                  fast_bn: bool = False, legalize_margin: int | None = 8):
    """Build the SPMD Bass program for `nl` images per core.

    fast_bn: gamma>0 and beta==0 (checked by the caller against the real
    inputs), so sign(bn(x)) == sign(x - mean): the binarize threshold
    needs only the channel means — the sum-of-squares pass and its share
    of the stats AllReduce are skipped entirely.
    """
    conv_dt = mybir.dt.float8e4 if use_fp8 else mybir.dt.bfloat16
    perf_mode = mybir.MatmulPerfMode.DoubleRow if use_fp8 else None

    # padded image rows: 1 guard row + nl*58 rows + tail guard, rounded so
    # that ROWS*58 (the DoubleRow j-step in bytes for fp8) is 16-aligned
    rows = 1 + nl * RPI + 1
    while (rows * RG) % 16 != 0:
        rows += 1

    cnt = nl * n_cores * HW  # BN reduction count per channel
    nst = 2 if fast_bn else 4  # stats vectors exchanged: sums (+ sumsqs)

    nc = bass.Bass(num_devices=n_cores)

    x_d = nc.declare_dram_parameter("x", [nl, C, H, W], F32, isOutput=False)
    g_d = nc.declare_dram_parameter("gamma", [C], F32, isOutput=False)
    be_d = nc.declare_dram_parameter("beta", [C], F32, isOutput=False)
    w_d = nc.declare_dram_parameter("weight", [O, C, KH, KW], F32, isOutput=False)
    bi_d = nc.declare_dram_parameter("bias", [O], F32, isOutput=False)
    out_d = nc.declare_dram_parameter("out", [nl, O, H, W], F32, isOutput=True)

    replica = [list(range(n_cores))]
    tr_dt = mybir.dt.bfloat16

    with tile.TileContext(nc) as tc:
        with (
            tc.tile_pool(name="consts", bufs=1) as consts,
            tc.tile_pool(name="xin", bufs=2 * nl) as xin_pool,
            tc.tile_pool(name="xbin", bufs=1) as xbin_pool,
            tc.tile_pool(name="wp", bufs=1) as wp,
            tc.tile_pool(name="stat", bufs=1) as stat,
            tc.tile_pool(name="psum", bufs=6, space="PSUM") as psum_pool,
            tc.tile_pool(name="psumt", bufs=2, space="PSUM") as psumt_pool,
            tc.tile_pool(name="osb", bufs=6) as osb_pool,
            tc.tile_pool(name="dram", bufs=1, space="DRAM") as dram_pool,
        ):
            # ---- x loads + local BN stats (issued first: critical path to
            # the AllReduce; everything else fills in around it) ----
            NCH = 4  # DMA/reduce chunks per (img, cg) tile
            xsum = stat.tile([P, 2, nl, NCH], F32)
            if not fast_bn:
                xss = stat.tile([P, 2, nl, NCH], F32)
            sq_scr = stat.tile([P, HW // NCH], F32, tag="sq_scr")
            xts = {}
            part = HW // NCH
            rch = H // NCH
            for img in range(nl):
                for cg in range(2):
                    xt = xin_pool.tile([P, H, W], F32, tag="xt")
                    xts[(img, cg)] = xt
                    xt_flat = xt.rearrange("p h w -> p (h w)")
                    # chunked DMAs: engage more DMA queues from t=0 and let
                    # the stats reductions chase at finer grain
                    for ch in range(NCH):
                        nc.sync.dma_start(
                            out=xt[:, ch * rch:(ch + 1) * rch, :],
                            in_=x_d[img, cg * P:(cg + 1) * P,
                                    ch * rch:(ch + 1) * rch, :],
                        )
                        nc.vector.tensor_reduce(
                            out=xsum[:, cg, img, ch:ch + 1],
                            in_=xt_flat[:, ch * part:(ch + 1) * part],
                            axis=AX.X, op=ALU.add,
                        )
                        if not fast_bn:
                            nc.scalar.activation(
                                out=sq_scr[:],
                                in_=xt_flat[:, ch * part:(ch + 1) * part],
                                func=AF.Square,
                                accum_out=xss[:, cg, img, ch:ch + 1],
                            )
            tloc = stat.tile([P, nst], F32)
            nc.vector.tensor_reduce(
                out=tloc[:, 0:2], in_=xsum[:], axis=AX.XY, op=ALU.add
            )
            if not fast_bn:
                nc.vector.tensor_reduce(
                    out=tloc[:, 2:4], in_=xss[:], axis=AX.XY, op=ALU.add
                )

            # ---- AllReduce of the stats vectors ----
            ar_in = dram_pool.tile([nst, P], F32)
            ar_out = dram_pool.tile([n_cores, nst, P], F32)
            nc.sync.dma_start(out=ar_in[:].rearrange("a p -> p a"), in_=tloc[:])
            if n_cores > 1:
                # AllGather + local sum: one collective phase instead of
                # the reduce+broadcast pair inside AllReduce
                nc.gpsimd.collective_compute(
                    "AllGather",
                    ALU.bypass,
                    replica_groups=replica,
                    ins=[ar_in[:]],
                    outs=[ar_out[:]],
                )
            else:
                nc.gpsimd.dma_start(out=ar_out[0], in_=ar_in[:])
            gath = stat.tile([P, n_cores * nst], F32)
            nc.sync.dma_start(
                out=gath[:].rearrange("p (r a) -> p r a", r=n_cores),
                in_=ar_out[:].rearrange("r a p -> p r a"),
            )
            gstat = stat.tile([P, nst], F32)
            nc.vector.tensor_reduce(
                out=gstat[:],
                in_=gath.rearrange("p (r a) -> p a r", r=n_cores),
                axis=AX.X, op=ALU.add,
            )

            # ---- parameter loads + weight prep (off the critical path) ----
            identity = consts.tile([P, P], tr_dt)
            make_identity(nc, identity)
            wT = wp.tile([P, 2, 18 * P], conv_dt)
            escale = consts.tile([P, 2], F32)   # m/2304 per (o_part, og)
            ebias = consts.tile([P, 2], F32)    # escale * bias
            bias_sb = consts.tile([P, 2], F32)
            nc.sync.dma_start(
                out=bias_sb[:], in_=bi_d[:].rearrange("(a p) -> p a", a=2, p=P)
            )
            gam2 = consts.tile([P, 2], F32)
            bet2 = consts.tile([P, 2], F32)
            if not fast_bn:
                nc.sync.dma_start(
                    out=gam2[:], in_=g_d[:].rearrange("(a p) -> p a", a=2, p=P)
                )
                nc.sync.dma_start(
                    out=bet2[:], in_=be_d[:].rearrange("(a p) -> p a", a=2, p=P)
                )

            for og in range(2):
                w_nat = wp.tile([P, CK], F32, tag=f"wnat{og}")
                nc.sync.dma_start(
                    out=w_nat[:], in_=w_d[og * P:(og + 1) * P, :, :, :]
                )
                wv = w_nat.rearrange("p (c k) -> p k c", c=C, k=NKK)
                kmean = stat.tile([P, NKK], F32, tag=f"kmean{og}")
                nc.vector.tensor_reduce(
                    out=kmean[:], in_=wv, axis=AX.X, op=ALU.add
                )
                nc.vector.tensor_scalar_mul(
                    out=kmean[:], in0=kmean[:], scalar1=1.0 / C
                )
                cent = wp.tile([P, CK], F32, tag=f"cent{og}")  # (k, c) layout
                for k in range(NKK):
                    nc.vector.tensor_scalar(
                        out=cent[:, k * C:(k + 1) * C],
                        in0=wv[:, k, :],
                        scalar1=kmean[:, k:k + 1],
                        scalar2=None,
                        op0=ALU.subtract,
                    )
                sgn = wp.tile([P, CK], tr_dt, tag=f"sgn{og}")
                nc.scalar.activation(out=sgn[:], in_=cent[:], func=AF.Sign)
                # clip to [-1,1] (into the now-dead w_nat), then sum |.|
                nc.vector.tensor_scalar(
                    out=w_nat[:], in0=cent[:],
                    scalar1=-1.0, scalar2=1.0, op0=ALU.max, op1=ALU.min,
                )
                mraw = stat.tile([P, 1], F32, tag=f"mraw{og}")
                nc.vector.tensor_reduce(
                    out=mraw[:], in_=w_nat[:], axis=AX.X, op=ALU.add,
                    apply_absolute_value=True,
                )
                nc.vector.tensor_scalar_mul(
                    out=escale[:, og:og + 1], in0=mraw[:], scalar1=1.0 / CK
                )
                # ebias = escale * bias on ACT (mixes DMA + DVE deps; DVE
                # tensor_tensor has a single sync-wait slot)
                nc.scalar.activation(
                    out=ebias[:, og:og + 1],
                    in_=bias_sb[:, og:og + 1],
                    func=AF.Copy,
                    scale=escale[:, og:og + 1],
                )
                # transpose each [o=128, c=128] block into [c, o]
                for k in range(NKK):
                    for cg in range(2):
                        pt = psumt_pool.tile([P, P], tr_dt, tag="pt")
                        nc.tensor.transpose(
                            out=pt[:],
                            in_=sgn[:, k * C + cg * P: k * C + (cg + 1) * P],
                            identity=identity[:],
                        )
                        nc.scalar.copy(
                            out=wT[:, cg, (k * 2 + og) * P:(k * 2 + og + 1) * P],
                            in_=pt[:],
                        )

            # ---- BN affine coefficients from the reduced stats ----
            mean = stat.tile([P, 2], F32)
            nc.vector.tensor_scalar_mul(
                out=mean[:], in0=gstat[:, 0:2], scalar1=1.0 / cnt
            )
            a_t = stat.tile([P, 2], F32)
            b_t = stat.tile([P, 2], F32)
            if fast_bn:
                # sign(bn(x)) == sign(x - mean): scale 1, bias = -mean
                nc.vector.tensor_scalar_mul(
                    out=b_t[:], in0=mean[:], scalar1=-1.0
                )
            else:
                var = stat.tile([P, 2], F32)
                nc.vector.tensor_mul(out=var[:], in0=mean[:], in1=mean[:])
                ex2 = stat.tile([P, 2], F32)
                nc.vector.tensor_scalar_mul(
                    out=ex2[:], in0=gstat[:, 2:4], scalar1=1.0 / cnt
                )
                nc.vector.tensor_sub(out=var[:], in0=ex2[:], in1=var[:])
                eps_ap = stat.tile([P, 1], F32)
                nc.vector.memset(eps_ap[:], BN_EPS)
                stdv = stat.tile([P, 2], F32)
                nc.scalar.activation(
                    out=stdv[:], in_=var[:], func=AF.Sqrt, bias=eps_ap[:]
                )
                rinv = stat.tile([P, 2], F32)
                nc.vector.reciprocal(out=rinv[:], in_=stdv[:])
                # a = gamma*rinv; b = beta - mean*a — on ACT (DMA+DVE deps)
                ma_t = stat.tile([P, 2], F32)
                for cg in range(2):
                    nc.scalar.activation(
                        out=a_t[:, cg:cg + 1], in_=rinv[:, cg:cg + 1],
                        func=AF.Copy, scale=gam2[:, cg:cg + 1],
                    )
                    nc.scalar.activation(
                        out=ma_t[:, cg:cg + 1], in_=mean[:, cg:cg + 1],
                        func=AF.Copy, scale=a_t[:, cg:cg + 1],
                    )
                    nc.scalar.activation(
                        out=b_t[:, cg:cg + 1], in_=ma_t[:, cg:cg + 1],
                        func=AF.Identity, scale=-1.0, bias=bet2[:, cg:cg + 1],
                    )

            # ---- binarize x + conv ----
            xbin = xbin_pool.tile([P, 2, rows, RG], conv_dt)
            nc.gpsimd.memset(xbin[:], 0.0)
            xflat = xbin.rearrange("p j r g -> p j (r g)")

            # PE warm-up: the PE sits idle through the stats/AllReduce
            # window, so HAM throttles it to 1.2 GHz and the first ~3.4us
            # of conv run at half speed. Issue throwaway matmuls gated on
            # the post-AllReduce b_t so they run right before the conv.
            if use_fp8:
                warm_rhs = wp.tile([P, 2, FREE], conv_dt)
                nc.gpsimd.memset(warm_rhs[:], 0.0)
                nc.scalar.activation(
                    out=warm_rhs[:, 0, 0:1],
                    in_=b_t[:, 0:1],
                    func=AF.Copy,
                    scale=0.0,
                )
                ps_warm = psumt_pool.tile([P, FREE], F32, tag="pt")
                for i in range(24):
                    nc.tensor.matmul(
                        ps_warm[:],
                        lhsT=wT[:, :, 0:P],
                        rhs=warm_rhs[:],
                        start=True,
                        stop=True,
                        perf_mode=perf_mode,
                    )

            for img in range(nl):
                r_img = 1 + img * RPI  # first padded row of this image
                for cg in range(2):
                    nc.scalar.activation(
                        out=xbin[:, cg, r_img + 1: r_img + 1 + H, 1:1 + W],
                        in_=xts[(img, cg)][:],
                        func=AF.Sign,
                        scale=1.0 if fast_bn else a_t[:, cg:cg + 1],
                        bias=b_t[:, cg:cg + 1],
                    )
                for og in range(2):
                    for t in range(NT):
                        # output tile: valid rows [8t, 8t+8) of this image;
                        # rhs slices skip the pad columns entirely (448-wide
                        # 4D APs instead of 464 contiguous)
                        ps = psum_pool.tile([P, OFREE], F32, tag="ps")
                        ki = 0
                        for dh in range(3):
                            for dw in range(3):
                                r0 = r_img + t * TROWS + dh
                                blk = ((dh * 3 + dw) * 2 + og) * P
                                if use_fp8:
                                    nc.tensor.matmul(
                                        ps[:],
                                        lhsT=wT[:, :, blk:blk + P],
                                        rhs=xbin[:, :, r0:r0 + TROWS, dw:dw + W],
                                        start=(ki == 0),
                                        stop=(ki == NKK - 1),
                                        perf_mode=perf_mode,
                                    )
                                else:
                                    for cg in range(2):
                                        nc.tensor.matmul(
                                            ps[:],
                                            lhsT=wT[:, cg, blk:blk + P],
                                            rhs=xbin[:, cg, r0:r0 + TROWS, dw:dw + W],
                                            start=(ki == 0 and cg == 0),
                                            stop=(ki == NKK - 1 and cg == 1),
                                        )
                                ki += 1
                        ob = osb_pool.tile([P, OFREE], F32, tag="ob")
                        nc.vector.tensor_scalar(
                            out=ob[:],
                            in0=ps[:],
                            scalar1=escale[:, og:og + 1],
                            scalar2=ebias[:, og:og + 1],
                            op0=ALU.mult,
                            op1=ALU.add,
                        )
                        nc.scalar.activation(
                            out=ob[:], in_=ob[:], func=AF.Relu,
                        )
                        nc.sync.dma_start(
                            out=out_d[img, og * P:(og + 1) * P,
                                      t * TROWS:(t + 1) * TROWS, :],
                            in_=ob.rearrange("p (r w) -> p r w", r=TROWS),
                        )

    _legalize_sync_waits(nc)
    return nc


def kernel(**inputs: np.ndarray) -> np.ndarray:
    from concourse.bass_utils import run_bass_kernel_spmd

    x = np.ascontiguousarray(inputs["x"], dtype=np.float32)
    gamma = np.ascontiguousarray(inputs["gamma"], dtype=np.float32)
    beta = np.ascontiguousarray(inputs["beta"], dtype=np.float32)
    weight = np.ascontiguousarray(inputs["weight"], dtype=np.float32)
    bias = np.ascontiguousarray(inputs["bias"], dtype=np.float32)

    n = x.shape[0]
    nl = n // N_CORES
    # sign(bn(x)) == sign(x - mean) whenever gamma > 0 and beta == 0 —
    # exact algebraic simplification for these inputs, checked here; the
    # general path handles anything else.
    fast_bn = bool(np.all(gamma > 0) and np.all(beta == 0))
    nc = build_program(nl, N_CORES, use_fp8=True, fast_bn=fast_bn)

    in_maps = []
    for core in range(N_CORES):
        in_maps.append({
            "x": x[core * nl:(core + 1) * nl],
            "gamma": gamma,
            "beta": beta,
            "weight": weight,
            "bias": bias,
        })
    res = run_bass_kernel_spmd(nc, in_maps, list(range(N_CORES)))
    out = np.concatenate([r["out"] for r in res.results], axis=0)
    return out.astype(np.float32)


if __name__ == "__main__":
    # smoke test with random data
    rng = np.random.default_rng(0)
    inputs = {
        "x": rng.standard_normal((32, C, H, W), dtype=np.float32),
        "gamma": np.ones((C,), np.float32),
        "beta": np.zeros((C,), np.float32),
        "weight": (rng.standard_normal((O, C, KH, KW)) * 0.1).astype(np.float32),
        "bias": (rng.standard_normal((O,)) * 0.01).astype(np.float32),
    }
    out = kernel(**inputs)
    print(out.shape, out.dtype, float(np.abs(out).max()))

